# revision 7
# baseline (speedup 1.0000x reference)
"""Trainium2 Bass kernel for nn_ActorNetSpiking (4-layer LIF SNN, T=50).

Contract: kernel(**inputs) takes FULL unsharded inputs (x:[4096,512,50] f32,
W1..W4/b1..b4, batch_size) and returns the FULL [4096,2] f32 output.

Strategy: pure data parallel over 8 NeuronCores (batch 4096 -> 512/core).
Host pre-transposes each core's x shard to [T, S, B]; activations live
transposed on-chip ([feature, batch]).

v3 design (on top of v2.3's +-1 spike / folded-state scheme):
  * x and W1 in bf16: halves the dominant x HBM/DMA traffic; matmul rate
    on PE is identical (1 cycle/row for bf16 and fp32r alike).
  * per-layer state tiles merged to [128, 2B] (both 128-neuron chunks side
    by side in the free dim); PSUM P tiles merged to [128, 2B] (2 banks).
  * elementwise work rebalanced across DVE/Pool/Act. Per layer-step:
      A:  SY' = alpha*SY + P          (STT, DVE; PSUM source, merged)
      mem update, reassociated as ME' = (beta*ME - s) + SY' so the first
      op has no dependency on this step's matmul:
        L1 (act3): t0 = beta*ME (Act copy-scale); t1 = t0 - s (Pool);
            ME' = t1 + SY' (Pool)
        L2/L3 (dve2): t1 = beta*ME - s (STT, DVE); ME' = t1 + SY' (Pool)
      D:  s' = Sign(ME' - thr2) per 128-chunk (Act, bias = per-neuron thr)
    Assignment HW-tuned via A/B timing: real gpsimd (Pool) TT costs ~2x
    the cost model, DVE ~1.39x, so the balance point keeps Pool at 4
    merged TTs/step. Measured per-rep exec: v2 535us -> v3b 427us.
  * layer 4 unchanged: transposed [batch-part, (bchunk, action)] free=8.
"""

import sys

sys.path.insert(0, "/opt/trn_rl_repo")

from contextlib import ExitStack

import numpy as np
import ml_dtypes

import concourse.bass as bass
import concourse.bacc as bacc
import concourse.tile as tile
from concourse import mybir

F32 = mybir.dt.float32
F32R = mybir.dt.float32r
BF16 = mybir.dt.bfloat16
ALU = mybir.AluOpType
ACT = mybir.ActivationFunctionType

ALPHA = 0.9
BETA = 0.85
THR = 1.0

N_CORES = 8
B_FULL = 4096
S = 512
H = 256
A = 2
T_FULL = 50
B = B_FULL // N_CORES  # 512 per core
BC = B // 128  # 4 batch chunks for transposed layer 4

_BUILD_COUNTER = [0]


def build_nc(T=T_FULL, reps=1, dummy_x=False, mem_eng=None):
    """Build the single-core Bass program (SPMD: same program on all cores).

    dummy_x=True replaces the x input with an internal (uninitialized) DRAM
    tensor - identical DMA/compute structure without the host upload; used
    only for timing builds.

    mem_eng: per-layer mem-update strategy override, dict {1:.., 2:.., 3:..}
    with values in {"pool3", "dve2", "act3"}. Default (L1=act3, L2/L3=dve2)
    is HW-tuned: real gpsimd TT runs ~2x the cost model, so keep Pool to
    one TT-add per dve2 layer plus L1's sub/add.
    """
    me = {1: "act3", 2: "dve2", 3: "dve2"}
    me.update(mem_eng or {})
    nc = bacc.Bacc(None, target_bir_lowering=False)

    MMDT = F32R
    if dummy_x:
        _BUILD_COUNTER[0] += 1
        xt = nc.dram_tensor(f"xt_dummy{_BUILD_COUNTER[0]}", [T, S, B], BF16)
    else:
        xt = nc.declare_dram_parameter("xt", [T, S, B], BF16, isOutput=False)
    w1t = nc.declare_dram_parameter("w1t", [S, H], BF16, isOutput=False)
    w2t = nc.declare_dram_parameter("w2t", [H, H], MMDT, isOutput=False)
    w3t = nc.declare_dram_parameter("w3t", [H, H], MMDT, isOutput=False)
    w4t = nc.declare_dram_parameter("w4t", [H, A], MMDT, isOutput=False)
    nthr = {
        L: nc.declare_dram_parameter(f"nthr{L}", [H, 1], F32, isOutput=False)
        for L in (1, 2, 3)
    }
    thr4 = nc.declare_dram_parameter("thr4", [128, 2 * BC], F32, isOutput=False)
    isyn = {
        L: nc.declare_dram_parameter(f"isyn{L}", [H, B], F32, isOutput=False)
        for L in (1, 2, 3)
    }
    imem = {
        L: nc.declare_dram_parameter(f"imem{L}", [H, B], F32, isOutput=False)
        for L in (1, 2, 3)
    }
    isyn4 = nc.declare_dram_parameter("isyn4", [128, 2 * BC], F32, isOutput=False)
    imem4 = nc.declare_dram_parameter("imem4", [128, 2 * BC], F32, isOutput=False)
    sinit = nc.declare_dram_parameter("sinit", [128, B], MMDT, isOutput=False)
    out = nc.declare_dram_parameter("out", [128, 2 * BC], F32, isOutput=True)

    KC1 = S // 128  # 4 k-chunks for layer 1
    KC = H // 128  # 2 k-chunks for layers 2-4
    MC = H // 128  # 2 m-chunks for layers 1-3
    B2 = 2 * B  # merged free size

    with tile.TileContext(nc) as tc, ExitStack() as ctx:
        wp = ctx.enter_context(tc.tile_pool(name="weights", bufs=1))
        xp = ctx.enter_context(tc.tile_pool(name="x", bufs=3))
        sp = ctx.enter_context(tc.tile_pool(name="state", bufs=2))
        tp = ctx.enter_context(tc.tile_pool(name="tmp", bufs=2))
        pp = ctx.enter_context(tc.tile_pool(name="psum", bufs=1, space="PSUM"))

        # --- load weights ---
        w1 = []
        for k in range(KC1):
            wt = wp.tile([128, H], BF16, tag=f"w1_{k}")
            nc.sync.dma_start(wt[:], w1t[k * 128 : (k + 1) * 128, :])
            w1.append(wt)
        w23 = {}
        for name, dram in (("w2", w2t), ("w3", w3t)):
            lst = []
            for k in range(KC):
                wt = wp.tile([128, H], MMDT, tag=f"{name}_{k}")
                nc.sync.dma_start(wt[:], dram[k * 128 : (k + 1) * 128, :])
                lst.append(wt)
            w23[name] = lst
        w4 = []
        for k in range(KC):
            wt = wp.tile([128, A], MMDT, tag=f"w4_{k}")
            nc.sync.dma_start(wt[:], w4t[k * 128 : (k + 1) * 128, :])
            w4.append(wt)

        # thresholds: negated [128,1] per chunk for Act bias; [128, 8] for L4
        nthr_t = {}
        for L in (1, 2, 3):
            for m in range(MC):
                t_ = wp.tile([128, 1], F32, tag=f"nthr{L}_{m}")
                nc.sync.dma_start(t_[:], nthr[L][m * 128 : (m + 1) * 128, :])
                nthr_t[(L, m)] = t_
        thr4_t = wp.tile([128, 2 * BC], F32, tag="thr4")
        nc.sync.dma_start(thr4_t[:], thr4[:])

        # beta broadcast tile for Pool TT-mult
        beta_t = wp.tile([128, B2], F32, tag="beta")
        nc.vector.memset(beta_t[:], BETA)

        # --- initial states (merged [128, 2B] per layer) ---
        state = {}
        for L in (1, 2, 3):
            st = sp.tile([128, B2], F32, tag=f"sy{L}")
            mt = sp.tile([128, B2], F32, tag=f"me{L}")
            pt = sp.tile([128, B2], MMDT, tag=f"s{L}")
            for m in range(MC):
                sl = slice(m * B, (m + 1) * B)
                nc.sync.dma_start(st[:, sl], isyn[L][m * 128 : (m + 1) * 128, :])
                nc.sync.dma_start(mt[:, sl], imem[L][m * 128 : (m + 1) * 128, :])
                nc.sync.dma_start(pt[:, sl], sinit[:, :])  # no-spike == -1
            state[("sy", L)] = st
            state[("me", L)] = mt
            state[("s", L)] = pt
        sy4 = sp.tile([128, 2 * BC], F32, tag="sy4")
        nc.sync.dma_start(sy4[:], isyn4[:])
        me4 = sp.tile([128, 2 * BC], F32, tag="me4")
        nc.sync.dma_start(me4[:], imem4[:])
        r4 = sp.tile([128, 2 * BC], F32, tag="r4")
        nc.vector.memset(r4[:], 0.0)
        acc = sp.tile([128, 2 * BC], F32, tag="acc")
        nc.vector.memset(acc[:], 0.0)
        state[("sy", 4)] = sy4
        state[("me", 4)] = me4
        state[("r", 4)] = r4

        def lif_update(L, ps):
            """Merged LIF update for layers 1-3. ps: merged [128, 2B] psum."""
            sy_o = state[("sy", L)]
            me_o = state[("me", L)]
            s_o = state[("s", L)]
            # A: SY' = alpha*SY + P   (DVE STT, merged, PSUM source)
            sy_n = sp.tile([128, B2], F32, tag=f"sy{L}")
            nc.vector.scalar_tensor_tensor(
                sy_n[:], sy_o[:], ALPHA, ps[:], op0=ALU.mult, op1=ALU.add
            )
            # mem update: ME' = (beta*ME - s) + SY'
            me_n = sp.tile([128, B2], F32, tag=f"me{L}")
            kind = me[L]
            if kind == "dve2":
                t1 = tp.tile([128, B2], F32, tag=f"t1_{L}")
                nc.vector.scalar_tensor_tensor(
                    t1[:], me_o[:], BETA, s_o[:].bitcast(F32),
                    op0=ALU.mult, op1=ALU.subtract,
                )
                nc.gpsimd.tensor_tensor(me_n[:], t1[:], sy_n[:], op=ALU.add)
            elif kind == "pool3":
                t0 = tp.tile([128, B2], F32, tag=f"t0_{L}")
                nc.gpsimd.tensor_tensor(t0[:], me_o[:], beta_t[:], op=ALU.mult)
                t1 = tp.tile([128, B2], F32, tag=f"t1_{L}")
                nc.gpsimd.tensor_tensor(
                    t1[:], t0[:], s_o[:].bitcast(F32), op=ALU.subtract
                )
                nc.gpsimd.tensor_tensor(me_n[:], t1[:], sy_n[:], op=ALU.add)
            elif kind == "act3":
                t0 = tp.tile([128, B2], F32, tag=f"t0_{L}")
                nc.scalar.mul(t0[:], me_o[:], BETA)
                t1 = tp.tile([128, B2], F32, tag=f"t1_{L}")
                nc.gpsimd.tensor_tensor(
                    t1[:], t0[:], s_o[:].bitcast(F32), op=ALU.subtract
                )
                nc.gpsimd.tensor_tensor(me_n[:], t1[:], sy_n[:], op=ALU.add)
            elif kind == "mix":
                # half 0 via DVE STT, half 1 via Act-mul + Pool-sub; one
                # merged Pool add. Balances DVE-STT halves across engines.
                h0, h1 = slice(0, B), slice(B, B2)
                t1 = tp.tile([128, B2], F32, tag=f"t1_{L}")
                nc.vector.scalar_tensor_tensor(
                    t1[:, h0], me_o[:, h0], BETA, s_o[:, h0].bitcast(F32),
                    op0=ALU.mult, op1=ALU.subtract,
                )
                t0 = tp.tile([128, B], F32, tag=f"t0_{L}")
                nc.scalar.mul(t0[:], me_o[:, h1], BETA)
                nc.gpsimd.tensor_tensor(
                    t1[:, h1], t0[:], s_o[:, h1].bitcast(F32), op=ALU.subtract
                )
                nc.gpsimd.tensor_tensor(me_n[:], t1[:], sy_n[:], op=ALU.add)
            else:
                raise ValueError(kind)
            # D: s' = Sign(ME' - thr2), per 128-neuron chunk (Act bias)
            s_n = sp.tile([128, B2], MMDT, tag=f"s{L}")
            for m in range(MC):
                sl = slice(m * B, (m + 1) * B)
                nc.scalar.activation(
                    s_n[:, sl], me_n[:, sl], ACT.Sign,
                    bias=nthr_t[(L, m)][:], scale=1.0,
                )
            state[("sy", L)] = sy_n
            state[("me", L)] = me_n
            state[("s", L)] = s_n

        # spike-tile history per layer (read by the next layer one iteration
        # later under the skewed pipeline; pool bufs=2 covers the lifetime)
        shist = {1: [], 2: [], 3: []}

        def emit_l1(t):
            xtiles = []
            for k in range(KC1):
                xt_k = xp.tile([128, B], BF16, tag=f"x_{k}")
                nc.sync.dma_start(xt_k[:], xt[t, k * 128 : (k + 1) * 128, :])
                xtiles.append(xt_k)
            ps = pp.tile([128, B2], F32, tag="ps1")
            for m in range(MC):
                for k in range(KC1):
                    nc.tensor.matmul(
                        ps[:, m * B : (m + 1) * B],
                        w1[k][:, m * 128 : (m + 1) * 128],
                        xtiles[k][:],
                        start=(k == 0),
                        stop=(k == KC1 - 1),
                    )
            lif_update(1, ps)
            shist[1].append(state[("s", 1)])

        def emit_l23(L, wname, t):
            sin = shist[L - 1][t]
            ps = pp.tile([128, B2], F32, tag=f"ps{L}")
            for m in range(MC):
                for k in range(KC):
                    nc.tensor.matmul(
                        ps[:, m * B : (m + 1) * B],
                        w23[wname][k][:, m * 128 : (m + 1) * 128],
                        sin[:, k * B : (k + 1) * B],
                        start=(k == 0),
                        stop=(k == KC - 1),
                    )
            lif_update(L, ps)
            shist[L].append(state[("s", L)])

        def emit_l4(t):
            nonlocal acc
            sin = shist[3][t]
            ps4 = pp.tile([128, 2 * BC], F32, tag="ps4")
            for c in range(BC):
                for k in range(KC):
                    nc.tensor.matmul(
                        ps4[:, 2 * c : 2 * c + 2],
                        sin[:, k * B + c * 128 : k * B + (c + 1) * 128],
                        w4[k][:],
                        start=(k == 0),
                        stop=(k == KC - 1),
                    )
            sy4_o, me4_o, r4_o = state[("sy", 4)], state[("me", 4)], state[("r", 4)]
            sy4_n = sp.tile([128, 2 * BC], F32, tag="sy4")
            nc.vector.scalar_tensor_tensor(
                sy4_n[:], sy4_o[:], ALPHA, ps4[:], op0=ALU.mult, op1=ALU.add
            )
            u4 = tp.tile([128, 2 * BC], F32, tag="u4")
            nc.vector.scalar_tensor_tensor(
                u4[:], me4_o[:], BETA, sy4_n[:], op0=ALU.mult, op1=ALU.add
            )
            me4_n = sp.tile([128, 2 * BC], F32, tag="me4")
            nc.gpsimd.tensor_tensor(me4_n[:], u4[:], r4_o[:], op=ALU.subtract)
            r4_n = sp.tile([128, 2 * BC], F32, tag="r4")
            nc.vector.tensor_tensor(r4_n[:], me4_n[:], thr4_t[:], op=ALU.is_gt)
            acc_n = sp.tile([128, 2 * BC], F32, tag="acc")
            nc.gpsimd.tensor_tensor(acc_n[:], acc[:], r4_n[:], op=ALU.add)
            state[("sy", 4)] = sy4_n
            state[("me", 4)] = me4_n
            state[("r", 4)] = r4_n
            acc = acc_n

        # skewed pipeline: iteration i runs L1@t=i, L2@t=i-1, L3@t=i-2,
        # L4@t=i-3 -- every cross-layer input comes from a prior iteration,
        # so the four layer chains schedule independently.
        # reps>1 re-runs the whole dynamics for in-NEFF timing builds only.
        for _rep in range(reps):
            for lst in shist.values():
                lst.clear()
            for i in range(T + 3):
                if i < T:
                    emit_l1(i)
                if 0 <= i - 1 < T:
                    emit_l23(2, "w2", i - 1)
                if 0 <= i - 2 < T:
                    emit_l23(3, "w3", i - 2)
                if 0 <= i - 3 < T:
                    emit_l4(i - 3)

        res = tp.tile([128, 2 * BC], F32, tag="res")
        nc.vector.tensor_copy(res[:], acc[:])
        nc.sync.dma_start(out[:], res[:])

    nc.finalize()
    return nc


def fold_params(W1, b1, W2, b2, W3, b3, W4, b4):
    """Host-side folding: +-1 spikes, steady-state shifts, x2 scaling.

    Device weights: w1 = 2*W1 (bf16); w2 = W2; w3 = W3 (x2 scale cancels the
    /2 of the +-1 encoding); w4 = W4/2 (layer 4 unscaled states).
    """
    f8 = np.float64
    out = {}
    Ws = {1: W1.astype(f8), 2: W2.astype(f8), 3: W3.astype(f8), 4: W4.astype(f8)}
    bs = {1: b1.astype(f8), 2: b2.astype(f8), 3: b3.astype(f8), 4: b4.astype(f8)}
    # +-1 encoding for spike inputs of layers 2..4
    beff = {1: bs[1]}
    for L in (2, 3, 4):
        beff[L] = bs[L] + 0.5 * Ws[L].sum(axis=1)
    # device weights
    out["w1t"] = np.ascontiguousarray(
        (2.0 * Ws[1]).T.astype(ml_dtypes.bfloat16)
    )
    out["w2t"] = np.ascontiguousarray(Ws[2].T.astype(np.float32))
    out["w3t"] = np.ascontiguousarray(Ws[3].T.astype(np.float32))
    out["w4t"] = np.ascontiguousarray((0.5 * Ws[4]).T.astype(np.float32))

    # layers 1-3: x2-scaled shifted states, +-1 own-spike reset (-0.5s - 0.5)
    for L in (1, 2, 3):
        syn_inf = beff[L] / (1.0 - ALPHA)
        mem_inf = (syn_inf - 0.5) / (1.0 - BETA)
        thr2 = 2.0 * (THR - mem_inf)
        out[f"nthr{L}"] = (-thr2[:, None]).astype(np.float32)
        out[f"isyn{L}"] = np.ascontiguousarray(
            np.broadcast_to((-2.0 * syn_inf[:, None]).astype(np.float32), (H, B))
        )
        out[f"imem{L}"] = np.ascontiguousarray(
            np.broadcast_to((-2.0 * mem_inf[:, None]).astype(np.float32), (H, B))
        )
    # layer 4: unscaled, 0/1 reset
    syn_inf4 = beff[4] / (1.0 - ALPHA)  # [A]
    mem_inf4 = syn_inf4 / (1.0 - BETA)
    thr4 = THR - mem_inf4  # [A]
    # transposed layout [128, (bchunk, action)]
    out["thr4"] = np.ascontiguousarray(
        np.broadcast_to(
            np.tile(thr4, BC)[None, :].astype(np.float32), (128, 2 * BC)
        )
    )
    out["isyn4"] = np.ascontiguousarray(
        np.broadcast_to(
            np.tile(-syn_inf4, BC)[None, :].astype(np.float32), (128, 2 * BC)
        )
    )
    out["imem4"] = np.ascontiguousarray(
        np.broadcast_to(
            np.tile(-mem_inf4, BC)[None, :].astype(np.float32), (128, 2 * BC)
        )
    )
    out["sinit"] = np.full((128, B), -1.0, np.float32)
    return out


def make_in_maps(x, W1, b1, W2, b2, W3, b3, W4, b4, T=T_FULL):
    """Shard + transpose full inputs into per-core input maps."""
    common = fold_params(W1, b1, W2, b2, W3, b3, W4, b4)
    in_maps = []
    for c in range(N_CORES):
        xs = x[c * B : (c + 1) * B, :, :T]  # [B, S, T]
        xtc = np.ascontiguousarray(
            xs.transpose(2, 1, 0).astype(ml_dtypes.bfloat16)
        )  # [T, S, B] bf16
        m = dict(common)
        m["xt"] = xtc
        in_maps.append(m)
    return in_maps


def assemble_output(results, T=T_FULL):
    """results: per-core dicts with 'out' [128, 2*BC] raw spike counts."""
    outs = []
    for c in range(N_CORES):
        acc = results[c]["out"]  # [128, (bchunk, action)]
        per = acc.reshape(128, BC, A).transpose(1, 0, 2).reshape(B, A)
        outs.append(per)
    full = np.concatenate(outs, axis=0)  # [4096, A]
    return (full / np.float32(T)).astype(np.float32)


_NC_CACHE = {}


def kernel(x, W1, b1, W2, b2, W3, b3, W4, b4, batch_size=None, **_):
    x = np.asarray(x, np.float32)
    args = [np.asarray(a, np.float32) for a in (W1, b1, W2, b2, W3, b3, W4, b4)]
    from concourse.bass_utils import run_bass_kernel_spmd

    key = "main"
    if key not in _NC_CACHE:
        _NC_CACHE[key] = build_nc()
    nc = _NC_CACHE[key]
    in_maps = make_in_maps(x, *args)
    res = run_bass_kernel_spmd(nc, in_maps, list(range(N_CORES)))
    return assemble_output(res.results)


if __name__ == "__main__":
    nc = build_nc(T=2)
    print("built ok")
